# revision 3
# baseline (speedup 1.0000x reference)
"""Trainium2 Bass kernel for nn_CustomBSplineLayer.

Math: out[b,o] = sum_{i,g} coeff[o,i,g] * w[o,i] * s_g(clip(x[b,i], -1, 1))
where s_g is a cubic B-spline basis on uniform knots (spacing h = 2/7,
centers linspace(-1, 15/7, 12), 8 basis functions, order 3).

Uniform-knot identity (truncated-power representation):
    s_g(x) = (1/6) * sum_{r=0..4} w5[r] * relu(t - (g+r))^3,  w5 = [1,-4,6,-4,1]
with t = (clip(x,-1,1)+1)/h = clip in [0, 7].  Planes V_q = relu(t-q)^3 are
shared across g (q = g+r), and relu(t-q)^3 == 0 for q >= 7 when t <= 7, so only
q = 0..6 contribute.  Folding the 5-tap conv into the coefficient side gives
    out[b,o] = sum_{q=0..6, i} V_q[b,i] * G2[(q,i), o]
i.e. ONE [B, 7*512] @ [7*512, 512] matmul after a cheap elementwise stage.

Device layout (data-parallel over batch, 8 cores x 1024 rows):
  - x is pre-transposed on host: xt [512 i, 1024 b] per core, i on partitions.
  - V planes computed per (q, i-block): [128, 1024] tiles, directly usable as
    matmul lhsT slices (K = (q,i) on partitions, M = 128 batch cols).
  - G2 [3584, 512] replicated, rhs tiles [128, 512].
  - PSUM [128 b, 512 o] accumulates 28 k-tiles; kt-outer / bb-inner loop keeps
    only ~8 V tiles live.
  - Most planes run the PE in float32r (full-rate); the large-magnitude low-q
    planes can be run in fp32 (4x slower, exact) for accuracy: FP32_Q.
"""

import numpy as np

import concourse.mybir as mybir
from concourse import bacc
import concourse.tile as tile
from concourse.bass_utils import run_bass_kernel_spmd

F32 = mybir.dt.float32
F32R = mybir.dt.float32r
AOT = mybir.AluOpType
ACTF = mybir.ActivationFunctionType

N_CORES = 8
BATCH, I, O, G = 8192, 512, 512, 8
BC = BATCH // N_CORES          # 1024 batch rows per core
Q = 7                          # truncated-power planes q = 0..6
IB = I // 128                  # 4 i-blocks
KT = Q * IB                    # 28 k-tiles of 128
NBB = BC // 128                # 8 batch blocks of 128
H = 2.0 / 7.0                  # knot spacing

# q planes whose matmuls run in exact fp32 (the rest in float32r).
FP32_Q = (0,)

_programs = {}


def _build_program(fp32_q):
    fp32_q = tuple(sorted(fp32_q))
    nfq = len(fp32_q)
    nc = bacc.Bacc("TRN2", target_bir_lowering=False, debug=False,
                   num_devices=N_CORES)
    xt_d = nc.dram_tensor("xt", [I, BC], F32, kind="ExternalInput").ap()
    g2r_d = nc.dram_tensor("g2r", [KT * 128, O], F32R, kind="ExternalInput").ap()
    if nfq:
        g2f_d = nc.dram_tensor("g2f", [nfq * IB * 128, O], F32,
                               kind="ExternalInput").ap()
    qb_d = nc.dram_tensor("qb", [128, 8], F32, kind="ExternalInput").ap()
    out_d = nc.dram_tensor("out", [BC, O], F32, kind="ExternalOutput").ap()

    with tile.TileContext(nc) as tc:
        with tc.tile_pool(name="g", bufs=1) as gpool, \
             tc.tile_pool(name="x", bufs=2) as xpool, \
             tc.tile_pool(name="tp", bufs=1) as tppool, \
             tc.tile_pool(name="v", bufs=1) as vpool, \
             tc.tile_pool(name="tmp", bufs=3) as tpool, \
             tc.tile_pool(name="o", bufs=4) as opool, \
             tc.tile_pool(name="ps", bufs=1, space="PSUM") as pspool:

            qb_s = gpool.tile([128, 8], F32)
            nc.sync.dma_start(out=qb_s[:], in_=qb_d[:])

            # x / t' stage: t' = min(3.5*x, 3.5); then V planes need only t'.
            tps = []
            for ib in range(IB):
                xs = xpool.tile([128, BC], F32, tag="x")
                nc.sync.dma_start(out=xs[:], in_=xt_d[ib * 128:(ib + 1) * 128, :])
                tp = tppool.tile([128, BC], F32, tag=f"tp{ib}")
                nc.vector.tensor_scalar(out=tp[:], in0=xs[:], scalar1=3.5,
                                        scalar2=3.5, op0=AOT.mult, op1=AOT.min)
                tps.append(tp)

            # coefficient tiles (rhs): [128, kt, 512]
            if nfq:
                g2f_s = gpool.tile([128, nfq * IB, O], F32)
                nc.sync.dma_start(
                    out=g2f_s[:],
                    in_=g2f_d.rearrange("(kt p) o -> p kt o", p=128))
            g2r_s = gpool.tile([128, KT, O], F32R)
            for q in range(Q):
                nc.sync.dma_start(
                    out=g2r_s[:, q * IB:(q + 1) * IB, :],
                    in_=g2r_d[q * IB * 128:(q + 1) * IB * 128, :].rearrange(
                        "(kt p) o -> p kt o", p=128))

            psums = [pspool.tile([128, O], F32, name=f"ps{bb}", tag=f"ps{bb}")
                      for bb in range(NBB)]

            # plane production + matmul chase, kt-major
            act_sq_budget = 16   # planes whose square runs on ACT (engine balance)
            n_act_sq = 0
            for q in range(Q):
                qq = float(q) - 3.5
                is_f32 = q in fp32_q
                for ib in range(IB):
                    kt = q * IB + ib
                    tp = tps[ib]
                    a = tpool.tile([128, BC], F32, tag="a")
                    nc.scalar.activation(a[:], tp[:], ACTF.Relu,
                                         bias=qb_s[:, q:q + 1], scale=1.0)
                    sq = tpool.tile([128, BC], F32, tag="sq")
                    if n_act_sq < act_sq_budget and (ib % 2 == 0):
                        nc.scalar.activation(sq[:], a[:], ACTF.Square)
                        n_act_sq += 1
                    else:
                        nc.vector.scalar_tensor_tensor(
                            out=sq[:], in0=tp[:], scalar=qq, in1=a[:],
                            op0=AOT.subtract, op1=AOT.mult)
                    v = vpool.tile([128, BC], F32 if is_f32 else F32R,
                                   tag="vf" if is_f32 else "vr",
                                   bufs=3 if is_f32 else 8)
                    nc.vector.scalar_tensor_tensor(
                        out=v[:], in0=tp[:], scalar=qq, in1=sq[:],
                        op0=AOT.subtract, op1=AOT.mult)
                    if is_f32:
                        fi = fp32_q.index(q)
                        rhs = g2f_s[:, fi * IB + ib, :]
                    else:
                        rhs = g2r_s[:, kt, :]
                    for bb in range(NBB):
                        nc.tensor.matmul(psums[bb][:],
                                         v[:, bb * 128:(bb + 1) * 128],
                                         rhs,
                                         start=(kt == 0), stop=(kt == KT - 1))

            for bb in range(NBB):
                o = opool.tile([128, O], F32, tag="o")
                nc.scalar.copy(o[:], psums[bb][:])
                nc.sync.dma_start(out=out_d[bb * 128:(bb + 1) * 128, :], in_=o[:])

    nc.compile()
    return nc


def _get_program(fp32_q=FP32_Q):
    key = tuple(sorted(fp32_q))
    if key not in _programs:
        _programs[key] = _build_program(key)
    return _programs[key]


def _host_prep(x, weights, coefficients, fp32_q=FP32_Q):
    x = np.ascontiguousarray(np.asarray(x, dtype=np.float32))
    weights = np.asarray(weights, dtype=np.float32)
    coefficients = np.asarray(coefficients, dtype=np.float32)

    # G2[(q,i), o] = sum_g w5[q-g]/6 * coeff[o,i,g] * weights[o,i]
    c2 = coefficients.astype(np.float64) * weights.astype(np.float64)[:, :, None]
    c2 = c2.transpose(2, 1, 0)                     # [G, I, O]
    w5 = np.array([1.0, -4.0, 6.0, -4.0, 1.0]) / 6.0
    g2 = np.zeros((Q, I, O), dtype=np.float64)
    for q in range(Q):
        for g in range(G):
            r = q - g
            if 0 <= r <= 4:
                g2[q] += w5[r] * c2[g]
    g2 = np.ascontiguousarray(g2.reshape(KT * 128, O).astype(np.float32))

    xt = np.ascontiguousarray(x.T)                 # [I, B]
    qb = np.tile((3.5 - np.arange(8, dtype=np.float32))[None, :], (128, 1))

    fp32_q = tuple(sorted(fp32_q))
    in_maps = []
    for c in range(N_CORES):
        m = {
            "xt": np.ascontiguousarray(xt[:, c * BC:(c + 1) * BC]),
            "g2r": g2,
            "qb": qb,
        }
        if fp32_q:
            m["g2f"] = np.ascontiguousarray(
                np.concatenate([g2[q * IB * 128:(q + 1) * IB * 128]
                                for q in fp32_q], axis=0))
        in_maps.append(m)
    return in_maps


def _run(x, weights, coefficients, fp32_q=FP32_Q, **spmd_kwargs):
    nc = _get_program(fp32_q)
    in_maps = _host_prep(x, weights, coefficients, fp32_q)
    res = run_bass_kernel_spmd(nc, in_maps, list(range(N_CORES)), **spmd_kwargs)
    out = np.concatenate([res.results[c]["out"] for c in range(N_CORES)], axis=0)
    return out.astype(np.float32), res


def kernel(x, weights, coefficients):
    out, _ = _run(x, weights, coefficients)
    return out
